# revision 95
# baseline (speedup 1.0000x reference)
"""Trainium2 Bass kernel: 3x3 SAME conv (stride 1), NCHW fp32.

Problem: image [32, 64, 112, 112] * weight [64, 64, 3, 3] + bias [64]
Sharding: data-parallel over batch across 8 NeuronCores (4 images each).

Per-core strategy (pixel-major matmuls):
  - Padded image stored at row pitch 113: each flat row r of a channel is
    [zero][img_row r-1 (112 px)], so one zero column is shared as the
    right pad of row h and the left pad of row h+1 (plus zero rows at
    r=0 and r=113).  Channel flat length 114*113 = 12882.
  - SBUF layout [128, L]: partition 64*s + cin holds channel cin's flat
    padded pixels shifted by s*113 (s=1 = one padded row down).  The two
    shifted copies make the matmul contraction dim K = 128 = (cin, shift).
  - GEMM orientation: lhsT (stationary) = image patch [K=128, M=128
    output pixels], rhs (moving) = weights [K=128, F=64 couts], out =
    PSUM [128 pixels, 64 couts].  Output pixel slot o = h*113 + w over
    the padded pitch (w=112 is garbage, dropped on host).
  - 6 accumulating matmuls per 128-pixel block cover the 9 taps:
    j=0..2: offset o+j      -> taps (kh=0,kw=j) on s=0 rows and
                               (kh=1,kw=j) on s=1 rows (K=128 pair);
    j=3..5: offset o+226+kw -> tap (kh=2,kw) on s=0 rows; the s=1 rows
                               see garbage data x zero weight rows.
  - 99 blocks per image (99*128 = 12672 >= 12656 slots); groups of 8
    blocks accumulate into one 2KB PSUM bank; DVE drains each bank with
    a cast to bf16.  Two consecutive groups share one stage tile and one
    output DMA (HBM order [n, pixel%128, block, cout]), halving the
    drain-DMA count so the serial HWDGE (~630ns each) stays off the
    critical path; the last image's drains alternate Act/SP queues.
  - Blend (see BLEND below): the last B blocks of images 1-3 load a
    second arrangement S = channel shifted by (2*PITCH + t), t in {0,1},
    turning the three kh=2 taps into one full-K pair matmul plus one
    single -- 5 matmuls per blended block.  Sized to spare DMA-bus time
    (the bus is ~100% busy mid-run): each blended block costs +91ns of
    input DMA and saves 26.7ns of PE.
  - Startup: the weight DMA is issued on the gpsimd (Pool) queue, whose
    software DGE runs in parallel with the SP queue's hardware DGE, so
    chunk0 (SP) and the weights overlap their ~1.3us launch+sem
    latencies.  A short Pool memset then feeds N_WARMUP dummy matmuls
    that hold the tensor engine busy through the ~3us p-state ramp until
    chunk0's semaphore lands (~3.5us); real matmuls then run at full
    clock (26.7ns per 64-cout matmul).
  - Input chunking: each chunk is its own SBUF tile (loaded with 384
    cols of overlap) so a block's matmuls only wait on the one chunk DMA
    that covers their [base, base+356] read window.  Image 0 uses a
    small->big cascade sized against the ~1.4us per-DMA launch+sem
    latency; later images prefetch during the previous image's compute
    and use coarse chunks.  chunk1 of image 0 issues on the Activation
    queue (still free: the first drain comes later) to dodge the shared
    HWDGE serialization.
"""

import numpy as np

import concourse.bass as bass
import concourse.mybir as mybir
import concourse.tile as tile
from concourse import bacc, bass_utils

N_CORES = 8
IMGS = 4  # images per core
CIN = 64
COUT = 64
H = 112
W = 112
PITCH = 113  # shared-pad row pitch
ROWS = H + 2  # 114 padded rows
CH_LEN = ROWS * PITCH  # 12882 padded flat length per channel
NPIX = H * PITCH  # 12656 output pixel slots (incl. 1 garbage col per row)
MBLK = 128  # output pixels per block (PSUM partitions)
NBLK = 99  # blocks per image (99*128 = 12672 >= 12656)
GROUPS = (8, 8, 8, 8, 8, 8, 8, 8, 8, 8, 8, 7, 4)  # blocks per PSUM bank
CHUNK_OV = 356  # overlap: max lhsT read span past a block base
# Image-0 cascade: (cols, queue) per chunk; queue 0=SP hwdge, 1=Act hwdge.
# Sized so chunk k's sem (xfer end + ~0.95us) lands before the PE finishes
# the blocks covered by chunks 0..k-1 (PE burns 160ns per 128-col block,
# the DMA bus supplies ~1.41 cols/ns, each launch costs ~1.3us serial).
CASCADE = ((576, 0), (512, 1), (896, 0), (1152, 0), (1792, 0), (2816, 0), (2560, 0), (2752, 0))
STEADY = ((2944, 0), (4480, 0), (5632, 0))
NCOLS = NBLK * MBLK + CHUNK_OV  # 13056 chunked cols per image
TOTAL_IN = CIN * CH_LEN + 768  # 825216: + zero tail (covers shifted reads)
OUT_ROW = NBLK * COUT  # 6336 out elems per partition-row per image
OUT_LEN = 128 * OUT_ROW  # 811008 per image
# matmul lhsT column offsets per block base: pairs (kh=0/1, kw) then
# singles (kh=2, kw) read via the s=0 rows at +2 padded rows.
TAP_OFFS = (0, 1, 2, 2 * PITCH, 2 * PITCH + 1, 2 * PITCH + 2)
N_WARMUP = 3  # PE p-state warm-up matmuls (512 cols each)
# Blend: the last BLEND[n] blocks of image n use a second SBUF
# arrangement S (partition 64*t + cin = channel shifted by 2*PITCH + t,
# t in {0,1}) so taps (2,0)+(2,1) become ONE full-K pair matmul read at
# the block base and (2,2) a single at +2 -- 5 matmuls/block instead of
# 6.  Bounded by spare DMA-bus time: each blended block costs +91ns of
# input DMA (128 S-cols) and saves 26.7ns of PE.  Image 0 does not
# blend: its cascade already fills the early bus.
BLEND = (0, 28, 28, 22)  # blended blocks per image (from the end)

F32 = mybir.dt.float32
BF16 = mybir.dt.bfloat16


def _bounds(sizes, cap=NCOLS):
    bs, ends, c = [], [], 0
    for sz in sizes:
        bs.append(c)
        c += sz
        ends.append(min(c + CHUNK_OV, cap))
    assert c >= NBLK * MBLK, c
    return tuple(bs), tuple(ends)


def _chunk_of(base, bs, ends, need=TAP_OFFS[-1] + MBLK):
    """Chunk index whose tile covers [base, base + need)."""
    for c in range(len(bs) - 1, -1, -1):
        if base >= bs[c]:
            assert base + need <= ends[c], base
            return c
    raise AssertionError(base)


def _ap(ap_obj, offset, dims):
    """Manual AP on the same tensor handle; dims = [[step, count], ...]."""
    return bass.AP(tensor=ap_obj.tensor, offset=offset, ap=dims)


def build_nc(n_imgs=IMGS):
    nc = bacc.Bacc(
        "TRN2",
        target_bir_lowering=False,
        debug=False,
        num_devices=N_CORES,
    )
    img_d = nc.dram_tensor("image_flat", [n_imgs, TOTAL_IN], BF16, kind="ExternalInput")
    wt_d = nc.dram_tensor("weight6", [128, 6 * COUT], BF16, kind="ExternalInput")
    wtx_d = nc.dram_tensor("weight_extra", [128, 2 * COUT], BF16, kind="ExternalInput")
    out_d = nc.dram_tensor("out", [n_imgs, OUT_LEN], BF16, kind="ExternalOutput")

    img_ap = img_d.ap()
    out_ap = out_d.ap()

    def plan_for(n):
        plan = CASCADE if n == 0 else STEADY
        blend_b = BLEND[n]
        if blend_b:
            # blended tail blocks read only [base, base+130): the final
            # chunk ends at the max of the last unblended block's +356
            # window and the last block's +130 window.
            blend_base = (NBLK - blend_b) * MBLK
            cap = max(blend_base - MBLK + TAP_OFFS[-1] + MBLK,
                      (NBLK - 1) * MBLK + 130)
        else:
            cap = NCOLS
        bs, ends = _bounds([s for s, _ in plan], cap)
        return plan, bs, ends

    with tile.TileContext(nc) as tc:
        with (
            tc.tile_pool(name="img0", bufs=len(CASCADE)) as img0_pool,
            tc.tile_pool(name="img", bufs=2 * len(STEADY)) as img_pool,
            tc.tile_pool(name="sarr", bufs=2) as s_pool,
            tc.tile_pool(name="wt", bufs=1) as wt_pool,
            tc.tile_pool(name="stage", bufs=10) as stage_pool,
            tc.tile_pool(name="psum", bufs=6, space="PSUM") as psum_pool,
            tc.tile_pool(name="wups", bufs=1, space="PSUM") as wups_pool,
        ):
            # Weights go via the gpsimd (Pool) queue: its software DGE runs
            # on the Pool engine, in parallel with chunk0's HWDGE on the SP
            # path, so both sems land ~1us earlier than serialized HWDGEs.
            wt_t = wt_pool.tile([128, 6 * COUT], BF16)
            nc.gpsimd.dma_start(wt_t[:], wt_d.ap()[:])

            # PE p-state warm-up: dummy matmuls over a zeroed scratch tile
            # keep the tensor engine busy through the clock ramp until the
            # first chunk's semaphore lands.  The memset runs on Pool right
            # after the weight descriptor generation.  Results go to a
            # scratch PSUM bank that is never read.
            if N_WARMUP:
                wu_sb = wt_pool.tile([128, 512], BF16)
                nc.gpsimd.memset(wu_sb[:], 0)
                wu_ps = wups_pool.tile([128, 512], F32)
                for _ in range(N_WARMUP):
                    nc.tensor.matmul(
                        wu_ps[:], wu_sb[:, :128], wu_sb[:],
                        start=True, stop=True, skip_group_check=True,
                    )

            for n in range(n_imgs):
                # partition 64*s + cin <- flat padded channel cin shifted by
                # s*113; one tile per chunk so matmul deps are per-chunk.
                plan, bs, ends = plan_for(n)
                queues = [q for _, q in plan]
                blend_b = BLEND[n]
                blend_base = (NBLK - blend_b) * MBLK
                def issue_s():
                    blend_ln = blend_b * MBLK + 132
                    st = s_pool.tile([128, blend_ln], BF16)
                    src = _ap(
                        img_ap,
                        n * TOTAL_IN + blend_base + 2 * PITCH,
                        [[1, 2], [CH_LEN, CIN], [1, blend_ln]],
                    )
                    nc.sync.dma_start(st[:], src)
                    return st

                chunks = []
                s_t = None
                for c in range(len(bs)):
                    ln = ends[c] - bs[c]
                    pool = img0_pool if n == 0 else img_pool
                    ch_t = pool.tile([128, ln], BF16)
                    eng = nc.scalar if queues[c] == 1 else nc.sync
                    src = _ap(
                        img_ap,
                        n * TOTAL_IN + bs[c],
                        [[PITCH, 2], [CH_LEN, CIN], [1, ln]],
                    )
                    eng.dma_start(ch_t[:], src)
                    chunks.append(ch_t)
                if n == 0:
                    # extra blend weights: needed first by image 1 block 79,
                    # issued after image 0's supply is fully queued.
                    wtx_t = wt_pool.tile([128, 2 * COUT], BF16)
                    nc.sync.dma_start(wtx_t[:], wtx_d.ap()[:])
                if blend_b and s_t is None:
                    s_t = issue_s()

                last = n == n_imgs - 1
                b0 = 0
                stg = None
                for gi, gsz in enumerate(GROUPS):
                    ps = psum_pool.tile([128, GROUPS[0] * COUT], F32)
                    for i in range(gsz):
                        base = (b0 + i) * MBLK
                        blended = blend_b and base >= blend_base
                        c = _chunk_of(base, bs, ends, 130 if blended else TAP_OFFS[-1] + MBLK)
                        loc = base - bs[c]
                        dst = ps[:, i * COUT : (i + 1) * COUT]
                        if blended:
                            locs = base - blend_base
                            for j in range(3):
                                nc.tensor.matmul(
                                    dst,
                                    chunks[c][:, loc + j : loc + j + MBLK],
                                    wt_t[:, j * COUT : (j + 1) * COUT],
                                    start=(j == 0), stop=False,
                                    skip_group_check=True,
                                )
                            nc.tensor.matmul(
                                dst, s_t[:, locs : locs + MBLK],
                                wtx_t[:, :COUT],
                                start=False, stop=False, skip_group_check=True,
                            )
                            nc.tensor.matmul(
                                dst, s_t[:, locs + 2 : locs + 2 + MBLK],
                                wtx_t[:, COUT:],
                                start=False, stop=True, skip_group_check=True,
                            )
                        else:
                            for j, off in enumerate(TAP_OFFS):
                                # kh=2 singles (j>=3) contract only the
                                # s=0 half: K=64 AP keeps them off the
                                # trimmed s=1 cols.
                                kk = 128 if j < 3 else CIN
                                nc.tensor.matmul(
                                    dst,
                                    chunks[c][:kk, loc + off : loc + off + MBLK],
                                    wt_t[:kk, j * COUT : (j + 1) * COUT],
                                    start=(j == 0),
                                    stop=(j == 5),
                                    skip_group_check=True,
                                )
                    # Two consecutive groups share one stage tile and one
                    # output DMA: halving the drain-DMA count keeps the
                    # serial HWDGE (~630ns each) off the critical path at
                    # the end of the kernel.
                    if stg is None:
                        stg = stage_pool.tile([128, 2 * GROUPS[0] * COUT], BF16)
                        stg_b0, stg_cols = b0, 0
                    nc.vector.tensor_scalar_add(
                        stg[:, stg_cols : stg_cols + gsz * COUT],
                        ps[:, : gsz * COUT], 0.0,
                    )
                    stg_cols += gsz * COUT
                    if gi % 2 == 1 or gi == len(GROUPS) - 1:
                        dst = _ap(
                            out_ap,
                            n * OUT_LEN + stg_b0 * COUT,
                            [[OUT_ROW, 128], [1, stg_cols]],
                        )
                        # drain DMAs issue from the (otherwise idle) Act
                        # queue: a drain waiting on its DVE sem must not
                        # block the SP queue where the next image's input
                        # loads issue.  The last image's drains alternate
                        # Act/SP (no later input loads; one queue alone
                        # issues only every ~667ns, which would backlog
                        # the final drains past the last matmul).
                        eng = nc.sync if last and (gi % 4 == 3 or gi == len(GROUPS) - 1) else nc.scalar
                        eng.dma_start(dst, stg[:, :stg_cols])
                        stg = None
                    b0 += gsz

    nc.compile()
    return nc


_NC_CACHE = {}


def _get_nc(n_imgs=IMGS):
    if n_imgs not in _NC_CACHE:
        _NC_CACHE[n_imgs] = build_nc(n_imgs)
    return _NC_CACHE[n_imgs]


def _prep_inputs(image, weight):
    import ml_dtypes

    bf16 = ml_dtypes.bfloat16
    image = np.asarray(image, dtype=np.float32)
    weight = np.asarray(weight, dtype=np.float32)
    n = image.shape[0]
    # pitch-113 padded layout: flat row r = [0][img row r-1], rows 0 and
    # 113 all-zero.
    pad = np.zeros((n, CIN, ROWS, PITCH), np.float32)
    pad[:, :, 1 : 1 + H, 1 : 1 + W] = image
    img_flat = np.zeros((n, TOTAL_IN), bf16)
    img_flat[:, : CIN * CH_LEN] = pad.reshape(n, CIN * CH_LEN).astype(bf16)
    # weight blocks [128, 6*COUT]: j=0..2 pairs (kh=0 lower / kh=1 upper
    # rows, kw=j); j=3..5 singles (kh=2, kw=j-3) on lower rows only.
    wt6 = np.zeros((128, 6, COUT), np.float32)
    for j in range(3):
        wt6[:CIN, j] = weight[:, :, 0, j].T
        wt6[CIN:, j] = weight[:, :, 1, j].T
        wt6[:CIN, 3 + j] = weight[:, :, 2, j].T
    # blend extra weights: col block 0 = S-pair (2,0) lower / (2,1) upper;
    # col block 1 = single (2,2) lower, zero upper.
    wtx = np.zeros((128, 2, COUT), np.float32)
    wtx[:CIN, 0] = weight[:, :, 2, 0].T
    wtx[CIN:, 0] = weight[:, :, 2, 1].T
    wtx[:CIN, 1] = weight[:, :, 2, 2].T
    return (
        img_flat,
        np.ascontiguousarray(wt6.reshape(128, 6 * COUT)).astype(bf16),
        np.ascontiguousarray(wtx.reshape(128, 2 * COUT)).astype(bf16),
    )


def run_cores(image, weight, bias, trace=False, **kw):
    """Shard over 8 cores, run, return (full_output, BassKernelResults)."""
    img_flat, wt6, wtx = _prep_inputs(image, weight)
    n = img_flat.shape[0]
    per = n // N_CORES
    assert per * N_CORES == n
    nc = _get_nc(per)
    in_maps = [
        {
            "image_flat": np.ascontiguousarray(img_flat[i * per : (i + 1) * per]),
            "weight6": wt6,
            "weight_extra": wtx,
        }
        for i in range(N_CORES)
    ]
    res = bass_utils.run_bass_kernel_spmd(
        nc, in_maps, core_ids=list(range(N_CORES)), trace=trace, **kw
    )
    outs = []
    bias32 = np.asarray(bias, dtype=np.float32)
    for i in range(N_CORES):
        arr = np.asarray(res.results[i]["out"]).reshape(per, 128, NBLK, COUT)
        # pixel slot o = block*128 + p  ->  [n, cout, o]
        pc = arr.transpose(0, 3, 2, 1).reshape(per, COUT, NBLK * 128)
        img = pc[:, :, :NPIX].reshape(per, COUT, H, PITCH)[:, :, :, :W]
        outs.append(img.astype(np.float32) + bias32[None, :, None, None])
    return np.concatenate(outs, axis=0), res


def kernel(image, weight, bias):
    out, _ = run_cores(image, weight, bias, trace=False)
    return out


# revision 100
# speedup vs baseline: 1.0002x; 1.0002x over previous
"""Trainium2 Bass kernel: 3x3 SAME conv (stride 1), NCHW fp32.

Problem: image [32, 64, 112, 112] * weight [64, 64, 3, 3] + bias [64]
Sharding: data-parallel over batch across 8 NeuronCores (4 images each).

Per-core strategy (pixel-major matmuls):
  - Padded image stored at row pitch 113: each flat row r of a channel is
    [zero][img_row r-1 (112 px)], so one zero column is shared as the
    right pad of row h and the left pad of row h+1 (plus zero rows at
    r=0 and r=113).  Channel flat length 114*113 = 12882.
  - SBUF layout [128, L]: partition 64*s + cin holds channel cin's flat
    padded pixels shifted by s*113 (s=1 = one padded row down).  The two
    shifted copies make the matmul contraction dim K = 128 = (cin, shift).
  - GEMM orientation: lhsT (stationary) = image patch [K=128, M=128
    output pixels], rhs (moving) = weights [K=128, F=64 couts], out =
    PSUM [128 pixels, 64 couts].  Output pixel slot o = h*113 + w over
    the padded pitch (w=112 is garbage, dropped on host).
  - 6 accumulating matmuls per 128-pixel block cover the 9 taps:
    j=0..2: offset o+j      -> taps (kh=0,kw=j) on s=0 rows and
                               (kh=1,kw=j) on s=1 rows (K=128 pair);
    j=3..5: offset o+226+kw -> tap (kh=2,kw) on s=0 rows; the s=1 rows
                               see garbage data x zero weight rows.
  - 99 blocks per image (99*128 = 12672 >= 12656 slots); groups of 8
    blocks accumulate into one 2KB PSUM bank; DVE drains each bank with
    a cast to bf16.  Two consecutive groups share one stage tile and one
    output DMA (HBM order [n, pixel%128, block, cout]), halving the
    drain-DMA count so the serial HWDGE (~630ns each) stays off the
    critical path; the last image's drains alternate Act/SP queues.
  - Blend (see BLEND below): the last B blocks of images 1-3 load a
    second arrangement S = channel shifted by (2*PITCH + t), t in {0,1},
    turning the three kh=2 taps into one full-K pair matmul plus one
    single -- 5 matmuls per blended block.  Sized to spare DMA-bus time
    (the bus is ~100% busy mid-run): each blended block costs +91ns of
    input DMA and saves 26.7ns of PE.
  - Startup: the weight DMA is issued on the gpsimd (Pool) queue, whose
    software DGE runs in parallel with the SP queue's hardware DGE, so
    chunk0 (SP) and the weights overlap their ~1.3us launch+sem
    latencies.  A short Pool memset then feeds N_WARMUP dummy matmuls
    that hold the tensor engine busy through the ~3us p-state ramp until
    chunk0's semaphore lands (~3.5us); real matmuls then run at full
    clock (26.7ns per 64-cout matmul).
  - Input chunking: each chunk is its own SBUF tile (loaded with 384
    cols of overlap) so a block's matmuls only wait on the one chunk DMA
    that covers their [base, base+356] read window.  Image 0 uses a
    small->big cascade sized against the ~1.4us per-DMA launch+sem
    latency; later images prefetch during the previous image's compute
    and use coarse chunks.  chunk1 of image 0 issues on the Activation
    queue (still free: the first drain comes later) to dodge the shared
    HWDGE serialization.
"""

import numpy as np

import concourse.bass as bass
import concourse.mybir as mybir
import concourse.tile as tile
from concourse import bacc, bass_utils

N_CORES = 8
IMGS = 4  # images per core
CIN = 64
COUT = 64
H = 112
W = 112
PITCH = 113  # shared-pad row pitch
ROWS = H + 2  # 114 padded rows
CH_LEN = ROWS * PITCH  # 12882 padded flat length per channel
NPIX = H * PITCH  # 12656 output pixel slots (incl. 1 garbage col per row)
MBLK = 128  # output pixels per block (PSUM partitions)
NBLK = 99  # blocks per image (99*128 = 12672 >= 12656)
GROUPS = (8, 8, 8, 8, 8, 8, 8, 8, 8, 8, 8, 7, 4)  # blocks per PSUM bank
CHUNK_OV = 356  # overlap: max lhsT read span past a block base
# Image-0 cascade: (cols, queue) per chunk; queue 0=SP hwdge, 1=Act hwdge.
# Sized so chunk k's sem (xfer end + ~0.95us) lands before the PE finishes
# the blocks covered by chunks 0..k-1 (PE burns 160ns per 128-col block,
# the DMA bus supplies ~1.41 cols/ns, each launch costs ~1.3us serial).
CASCADE = ((576, 0), (512, 1), (896, 0), (1152, 0), (1792, 0), (2816, 0), (2560, 0), (2752, 0))
STEADY = ((2944, 0), (4352, 0), (5760, 0))
NCOLS = NBLK * MBLK + CHUNK_OV  # 13056 chunked cols per image
TOTAL_IN = CIN * CH_LEN + 768  # 825216: + zero tail (covers shifted reads)
OUT_ROW = NBLK * COUT  # 6336 out elems per partition-row per image
OUT_LEN = 128 * OUT_ROW  # 811008 per image
# matmul lhsT column offsets per block base: pairs (kh=0/1, kw) then
# singles (kh=2, kw) read via the s=0 rows at +2 padded rows.
TAP_OFFS = (0, 1, 2, 2 * PITCH, 2 * PITCH + 1, 2 * PITCH + 2)
N_WARMUP = 3  # PE p-state warm-up matmuls (512 cols each)
# Blend: the last BLEND[n] blocks of image n use a second SBUF
# arrangement S (partition 64*t + cin = channel shifted by 2*PITCH + t,
# t in {0,1}) so taps (2,0)+(2,1) become ONE full-K pair matmul read at
# the block base and (2,2) a single at +2 -- 5 matmuls/block instead of
# 6.  Bounded by spare DMA-bus time: each blended block costs +91ns of
# input DMA (128 S-cols) and saves 26.7ns of PE.  Image 0 does not
# blend: its cascade already fills the early bus.
BLEND = (0, 28, 28, 22)  # blended blocks per image (from the end)

F32 = mybir.dt.float32
BF16 = mybir.dt.bfloat16


def _bounds(sizes, cap=NCOLS):
    bs, ends, c = [], [], 0
    for sz in sizes:
        bs.append(c)
        c += sz
        ends.append(min(c + CHUNK_OV, cap))
    assert c >= NBLK * MBLK, c
    return tuple(bs), tuple(ends)


def _chunk_of(base, bs, ends, need=TAP_OFFS[-1] + MBLK):
    """Chunk index whose tile covers [base, base + need)."""
    for c in range(len(bs) - 1, -1, -1):
        if base >= bs[c]:
            assert base + need <= ends[c], base
            return c
    raise AssertionError(base)


def _ap(ap_obj, offset, dims):
    """Manual AP on the same tensor handle; dims = [[step, count], ...]."""
    return bass.AP(tensor=ap_obj.tensor, offset=offset, ap=dims)


def build_nc(n_imgs=IMGS):
    nc = bacc.Bacc(
        "TRN2",
        target_bir_lowering=False,
        debug=False,
        num_devices=N_CORES,
    )
    img_d = nc.dram_tensor("image_flat", [n_imgs, TOTAL_IN], BF16, kind="ExternalInput")
    wt_d = nc.dram_tensor("weight6", [128, 6 * COUT], BF16, kind="ExternalInput")
    wtx_d = nc.dram_tensor("weight_extra", [128, 2 * COUT], BF16, kind="ExternalInput")
    out_d = nc.dram_tensor("out", [n_imgs, OUT_LEN], BF16, kind="ExternalOutput")

    img_ap = img_d.ap()
    out_ap = out_d.ap()

    def plan_for(n):
        plan = CASCADE if n == 0 else STEADY
        blend_b = BLEND[n]
        if blend_b:
            # blended tail blocks read only [base, base+130): the final
            # chunk ends at the max of the last unblended block's +356
            # window and the last block's +130 window.
            blend_base = (NBLK - blend_b) * MBLK
            cap = max(blend_base - MBLK + TAP_OFFS[-1] + MBLK,
                      (NBLK - 1) * MBLK + 130)
        else:
            cap = NCOLS
        bs, ends = _bounds([s for s, _ in plan], cap)
        return plan, bs, ends

    with tile.TileContext(nc) as tc:
        with (
            tc.tile_pool(name="img0", bufs=len(CASCADE)) as img0_pool,
            tc.tile_pool(name="img", bufs=2 * len(STEADY)) as img_pool,
            tc.tile_pool(name="sarr", bufs=2) as s_pool,
            tc.tile_pool(name="wt", bufs=1) as wt_pool,
            tc.tile_pool(name="stage", bufs=10) as stage_pool,
            tc.tile_pool(name="psum", bufs=6, space="PSUM") as psum_pool,
            tc.tile_pool(name="wups", bufs=1, space="PSUM") as wups_pool,
        ):
            # Weights go via the gpsimd (Pool) queue: its software DGE runs
            # on the Pool engine, in parallel with chunk0's HWDGE on the SP
            # path, so both sems land ~1us earlier than serialized HWDGEs.
            wt_t = wt_pool.tile([128, 6 * COUT], BF16)
            nc.gpsimd.dma_start(wt_t[:], wt_d.ap()[:])

            # PE p-state warm-up: dummy matmuls over a zeroed scratch tile
            # keep the tensor engine busy through the clock ramp until the
            # first chunk's semaphore lands.  The memset runs on Pool right
            # after the weight descriptor generation.  Results go to a
            # scratch PSUM bank that is never read.
            if N_WARMUP:
                wu_sb = wt_pool.tile([128, 512], BF16)
                nc.gpsimd.memset(wu_sb[:], 0)
                wu_ps = wups_pool.tile([128, 512], F32)
                for _ in range(N_WARMUP):
                    nc.tensor.matmul(
                        wu_ps[:], wu_sb[:, :128], wu_sb[:],
                        start=True, stop=True, skip_group_check=True,
                    )

            for n in range(n_imgs):
                # partition 64*s + cin <- flat padded channel cin shifted by
                # s*113; one tile per chunk so matmul deps are per-chunk.
                plan, bs, ends = plan_for(n)
                queues = [q for _, q in plan]
                blend_b = BLEND[n]
                blend_base = (NBLK - blend_b) * MBLK
                def issue_s():
                    blend_ln = blend_b * MBLK + 132
                    st = s_pool.tile([128, blend_ln], BF16)
                    src = _ap(
                        img_ap,
                        n * TOTAL_IN + blend_base + 2 * PITCH,
                        [[1, 2], [CH_LEN, CIN], [1, blend_ln]],
                    )
                    nc.sync.dma_start(st[:], src)
                    return st

                chunks = []
                s_t = None
                for c in range(len(bs)):
                    ln = ends[c] - bs[c]
                    pool = img0_pool if n == 0 else img_pool
                    ch_t = pool.tile([128, ln], BF16)
                    eng = nc.scalar if queues[c] == 1 else nc.sync
                    src = _ap(
                        img_ap,
                        n * TOTAL_IN + bs[c],
                        [[PITCH, 2], [CH_LEN, CIN], [1, ln]],
                    )
                    eng.dma_start(ch_t[:], src)
                    chunks.append(ch_t)
                if n == 0:
                    # extra blend weights: needed first by image 1 block 79,
                    # issued after image 0's supply is fully queued.
                    wtx_t = wt_pool.tile([128, 2 * COUT], BF16)
                    nc.sync.dma_start(wtx_t[:], wtx_d.ap()[:])
                if blend_b and s_t is None:
                    s_t = issue_s()

                last = n == n_imgs - 1
                b0 = 0
                stg = None
                for gi, gsz in enumerate(GROUPS):
                    ps = psum_pool.tile([128, GROUPS[0] * COUT], F32)
                    for i in range(gsz):
                        base = (b0 + i) * MBLK
                        blended = blend_b and base >= blend_base
                        c = _chunk_of(base, bs, ends, 130 if blended else TAP_OFFS[-1] + MBLK)
                        loc = base - bs[c]
                        dst = ps[:, i * COUT : (i + 1) * COUT]
                        if blended:
                            locs = base - blend_base
                            for j in range(3):
                                nc.tensor.matmul(
                                    dst,
                                    chunks[c][:, loc + j : loc + j + MBLK],
                                    wt_t[:, j * COUT : (j + 1) * COUT],
                                    start=(j == 0), stop=False,
                                    skip_group_check=True,
                                )
                            nc.tensor.matmul(
                                dst, s_t[:, locs : locs + MBLK],
                                wtx_t[:, :COUT],
                                start=False, stop=False, skip_group_check=True,
                            )
                            nc.tensor.matmul(
                                dst, s_t[:, locs + 2 : locs + 2 + MBLK],
                                wtx_t[:, COUT:],
                                start=False, stop=True, skip_group_check=True,
                            )
                        else:
                            for j, off in enumerate(TAP_OFFS):
                                # kh=2 singles (j>=3) contract only the
                                # s=0 half: K=64 AP keeps them off the
                                # trimmed s=1 cols.
                                kk = 128 if j < 3 else CIN
                                nc.tensor.matmul(
                                    dst,
                                    chunks[c][:kk, loc + off : loc + off + MBLK],
                                    wt_t[:kk, j * COUT : (j + 1) * COUT],
                                    start=(j == 0),
                                    stop=(j == 5),
                                    skip_group_check=True,
                                )
                    # Two consecutive groups share one stage tile and one
                    # output DMA: halving the drain-DMA count keeps the
                    # serial HWDGE (~630ns each) off the critical path at
                    # the end of the kernel.
                    if stg is None:
                        stg = stage_pool.tile([128, 2 * GROUPS[0] * COUT], BF16)
                        stg_b0, stg_cols = b0, 0
                    nc.vector.tensor_scalar_add(
                        stg[:, stg_cols : stg_cols + gsz * COUT],
                        ps[:, : gsz * COUT], 0.0,
                    )
                    stg_cols += gsz * COUT
                    if gi % 2 == 1 or gi == len(GROUPS) - 1:
                        dst = _ap(
                            out_ap,
                            n * OUT_LEN + stg_b0 * COUT,
                            [[OUT_ROW, 128], [1, stg_cols]],
                        )
                        # drain DMAs issue from the (otherwise idle) Act
                        # queue: a drain waiting on its DVE sem must not
                        # block the SP queue where the next image's input
                        # loads issue.  The last image's drains alternate
                        # Act/SP (no later input loads; one queue alone
                        # issues only every ~667ns, which would backlog
                        # the final drains past the last matmul).
                        eng = nc.sync if last and (gi % 4 == 3 or gi == len(GROUPS) - 1) else nc.scalar
                        eng.dma_start(dst, stg[:, :stg_cols])
                        stg = None
                    b0 += gsz

    nc.compile()
    return nc


_NC_CACHE = {}


def _get_nc(n_imgs=IMGS):
    if n_imgs not in _NC_CACHE:
        _NC_CACHE[n_imgs] = build_nc(n_imgs)
    return _NC_CACHE[n_imgs]


def _prep_inputs(image, weight):
    import ml_dtypes

    bf16 = ml_dtypes.bfloat16
    image = np.asarray(image, dtype=np.float32)
    weight = np.asarray(weight, dtype=np.float32)
    n = image.shape[0]
    # pitch-113 padded layout: flat row r = [0][img row r-1], rows 0 and
    # 113 all-zero.
    pad = np.zeros((n, CIN, ROWS, PITCH), np.float32)
    pad[:, :, 1 : 1 + H, 1 : 1 + W] = image
    img_flat = np.zeros((n, TOTAL_IN), bf16)
    img_flat[:, : CIN * CH_LEN] = pad.reshape(n, CIN * CH_LEN).astype(bf16)
    # weight blocks [128, 6*COUT]: j=0..2 pairs (kh=0 lower / kh=1 upper
    # rows, kw=j); j=3..5 singles (kh=2, kw=j-3) on lower rows only.
    wt6 = np.zeros((128, 6, COUT), np.float32)
    for j in range(3):
        wt6[:CIN, j] = weight[:, :, 0, j].T
        wt6[CIN:, j] = weight[:, :, 1, j].T
        wt6[:CIN, 3 + j] = weight[:, :, 2, j].T
    # blend extra weights: col block 0 = S-pair (2,0) lower / (2,1) upper;
    # col block 1 = single (2,2) lower, zero upper.
    wtx = np.zeros((128, 2, COUT), np.float32)
    wtx[:CIN, 0] = weight[:, :, 2, 0].T
    wtx[CIN:, 0] = weight[:, :, 2, 1].T
    wtx[:CIN, 1] = weight[:, :, 2, 2].T
    return (
        img_flat,
        np.ascontiguousarray(wt6.reshape(128, 6 * COUT)).astype(bf16),
        np.ascontiguousarray(wtx.reshape(128, 2 * COUT)).astype(bf16),
    )


def run_cores(image, weight, bias, trace=False, **kw):
    """Shard over 8 cores, run, return (full_output, BassKernelResults)."""
    img_flat, wt6, wtx = _prep_inputs(image, weight)
    n = img_flat.shape[0]
    per = n // N_CORES
    assert per * N_CORES == n
    nc = _get_nc(per)
    in_maps = [
        {
            "image_flat": np.ascontiguousarray(img_flat[i * per : (i + 1) * per]),
            "weight6": wt6,
            "weight_extra": wtx,
        }
        for i in range(N_CORES)
    ]
    res = bass_utils.run_bass_kernel_spmd(
        nc, in_maps, core_ids=list(range(N_CORES)), trace=trace, **kw
    )
    outs = []
    bias32 = np.asarray(bias, dtype=np.float32)
    for i in range(N_CORES):
        arr = np.asarray(res.results[i]["out"]).reshape(per, 128, NBLK, COUT)
        # pixel slot o = block*128 + p  ->  [n, cout, o]
        pc = arr.transpose(0, 3, 2, 1).reshape(per, COUT, NBLK * 128)
        img = pc[:, :, :NPIX].reshape(per, COUT, H, PITCH)[:, :, :, :W]
        outs.append(img.astype(np.float32) + bias32[None, :, None, None])
    return np.concatenate(outs, axis=0), res


def kernel(image, weight, bias):
    out, _ = run_cores(image, weight, bias, trace=False)
    return out
